# revision 23
# baseline (speedup 1.0000x reference)
"""Trainium2 Bass kernel for causal multi-head attention with RoPE.

Problem: B=4, S=2048, D=768, H=12, HD=64 (torch-Linear style projections,
rotary embeddings on q/k, causal softmax, output projection + bias).

Sharding across 8 NeuronCores: core c handles batch c//2 and head-group
c%2 (6 of 12 heads). Each core computes a partial output projection
(its heads' contribution to ctx @ Wo.T); the host sums the two partials
per batch and adds the bias. No device collectives.

Per-core kernel (all matmul operands bf16, fp32 PSUM accumulation):
  - Q^T/K^T [hd, S] via pre-transposed weights. RoPE: rotate_half is a
    partition permutation done by 4 small PSUM->SBUF DMAs; then two
    full-partition DVE multiplies (cos, sign-folded sin) and a GpSimd add.
  - V [S, hd] per head with an appended ones column (row 64 of the PV
    accumulator becomes the softmax denominator for free).
  - Attention in 512-query windows, per head-pair: scores for heads A and
    B issue back-to-back with stationaries in PE row groups 0-63/64-127,
    so the two matmuls run concurrently (row tiling). One exp covers both
    heads' PSUM banks via a [128, 2, w] access pattern (scale=1/8 folded
    in, no max subtraction; scores are bounded). PV runs one k-chunk
    behind the exp stream so PE's in-order queue never stalls on ScalarE.
  - Denominator row is DMA-broadcast to 64 partitions, reciprocal'd with
    64 active lanes, and multiplied into the evicted ctx tile.
  - out = ctx^T-chunks.T @ Wo^T-chunks, interleaved with the tail of the
    attention so PE never idles; partial output summed on host.
"""

import numpy as np

B, S, D, H = 4, 2048, 768, 12
HD = D // H          # 64
N_CORES = 8
HEADS_PER_CORE = 6
PAIRS = 3            # head pairs per core
DC = D // 128        # 6 contraction chunks
NJ = S // 128        # 16 k-chunks
W = 512              # q-window width
NW = S // W          # 4 windows

_CACHE = {}


def _rope_tables():
    inv_freq = 1.0 / (10000.0 ** (np.arange(0, HD, 2, dtype=np.float64) / HD))
    ang = np.arange(S, dtype=np.float64)[:, None] * inv_freq[None, :]  # [S, 32]
    cos = np.cos(ang).astype(np.float32)   # [S, 32]
    sin = np.sin(ang).astype(np.float32)
    cosF = np.empty((128, S), np.float32)
    sinM = np.empty((128, S), np.float32)
    for g in range(4):
        cosF[32 * g:32 * g + 32] = cos.T
        sgn = -1.0 if g % 2 == 0 else 1.0
        sinM[32 * g:32 * g + 32] = sgn * sin.T
    return cosF, sinM


def _build_program(reps=1, dbg=False, ablate=()):
    import concourse.bacc as bacc
    import concourse.mybir as mybir
    import concourse.tile as tile

    f32 = mybir.dt.float32
    f32r = mybir.dt.float32r
    bf16 = mybir.dt.bfloat16
    AF = mybir.ActivationFunctionType
    OP = mybir.AluOpType

    nc = bacc.Bacc("TRN2", target_bir_lowering=False, debug=False,
                   num_devices=N_CORES)

    eT = nc.declare_dram_parameter("eT", [D, S], bf16, isOutput=False)
    wq = nc.declare_dram_parameter("wq", [D, 384], bf16, isOutput=False)
    wk = nc.declare_dram_parameter("wk", [D, 384], bf16, isOutput=False)
    wqr = nc.declare_dram_parameter("wqr", [D, 384], bf16, isOutput=False)
    wkr = nc.declare_dram_parameter("wkr", [D, 384], bf16, isOutput=False)
    wv = nc.declare_dram_parameter("wv", [D, 384], bf16, isOutput=False)
    wo = nc.declare_dram_parameter("wo", [384, D], bf16, isOutput=False)
    cosF_d = nc.declare_dram_parameter("cosF", [128, S], bf16, isOutput=False)
    sinM_d = nc.declare_dram_parameter("sinM", [128, S], bf16, isOutput=False)
    mask_d = nc.declare_dram_parameter("mask", [128, 128], bf16, isOutput=False)
    idn_d = nc.declare_dram_parameter("idn", [128, 128], bf16, isOutput=False)
    o = nc.declare_dram_parameter("o", [S, D], f32, isOutput=True)
    if dbg:
        qtd = nc.declare_dram_parameter("qtd", [128, PAIRS, S], bf16,
                                        isOutput=True)
        ktd = nc.declare_dram_parameter("ktd", [128, PAIRS, S], bf16,
                                        isOutput=True)
        vtd = nc.declare_dram_parameter("vtd", [128, NJ, HEADS_PER_CORE,
                                                HD + 1], bf16, isOutput=True)
        cxtd = nc.declare_dram_parameter("cxtd", [128, PAIRS, S], bf16,
                                         isOutput=True)

    with tile.TileContext(nc) as tc, \
            nc.allow_low_precision(reason="bf16 matmul operand tiles"):
        with tc.tile_pool(name="const", bufs=1) as cp:
            cosF = cp.tile([128, S], bf16)
            sinM = cp.tile([128, S], bf16)
            msk = cp.tile([128, 128], bf16)
            idn = cp.tile([128, 128], bf16)

            qt = cp.tile([128, PAIRS, S], bf16)
            # k stationaries, zero-padded per head so score matmuls can use
            # full 128-partition operands (half-partition streams are ~5x
            # slower); zeros null the other head's q rows
            ktA = cp.tile([128, PAIRS, S], bf16)
            ktB = cp.tile([128, PAIRS, S], bf16)
            nc.vector.memset(ktA[HD:128, :, :].bitcast(mybir.dt.uint16), 0)
            nc.vector.memset(ktB[0:HD, :, :].bitcast(mybir.dt.uint16), 0)
            vt = cp.tile([128, NJ, HEADS_PER_CORE, HD + 1], f32r)
            nc.vector.memset(vt[:, :, :, HD].bitcast(mybir.dt.uint32),
                             0x3F800000)
            cxt = cp.tile([128, PAIRS, S], bf16)
            if "noexp" in ablate:
                etc_ = cp.tile([128, 2, W], f32r)
                nc.vector.memset(etc_[:].bitcast(mybir.dt.uint32),
                                 0x3F000000)
            if "noattn" in ablate:
                nc.vector.memset(cxt[:].bitcast(mybir.dt.uint16), 0x3F00)
            if "noproj" in ablate:
                nc.vector.memset(qt[:].bitcast(mybir.dt.uint16), 0x3F00)
                nc.vector.memset(ktA[0:HD].bitcast(mybir.dt.uint16), 0x3F00)
                nc.vector.memset(ktB[HD:128].bitcast(mybir.dt.uint16), 0x3F00)
            wot = cp.tile([128, PAIRS, D], bf16)

            eT_r = eT[:].rearrange("(n p) s -> p n s", p=128)

            for _rep in range(reps):
                with (
                    tc.tile_pool(name="asb", bufs=3) as asb,
                    tc.tile_pool(name="projsb", bufs=1) as pjs,
                ):
                    scp_cm = tc.tile_pool(name="scp", bufs=2, space="PSUM")
                    scp = scp_cm.__enter__()
                    cxp_cm = tc.tile_pool(name="cxp", bufs=2, space="PSUM")
                    cxp = cxp_cm.__enter__()
                    pps_cm = tc.tile_pool(name="pps", bufs=2, space="PSUM")
                    pps = pps_cm.__enter__()

                    _DONE = object()

                    def consume(filler, n):
                        if filler is None:
                            return
                        for _ in range(n):
                            if next(filler, _DONE) is _DONE:
                                return

                    def proj_chunk(pair, cc, with_v, wqt, wkt, wvt):
                        """One 512-col chunk of Q^T/K^T (+V when with_v),
                        as a generator of small PE quanta so it can be
                        interleaved between attention steps."""
                        if "noproj" in ablate:
                            return
                        cols = slice(512 * cc, 512 * cc + 512)
                        etAB = pjs.tile([128, 6, 512], bf16, tag="et",
                                        bufs=3, name=f"eA{pair}{cc}")
                        nc.sync.dma_start(etAB[:], eT_r[:, :, cols])

                        def et(d):
                            return etAB[:, d, :]

                        for wt, wrt, dst in ((wqt, wqrt, qt),
                                             (wkt, wkrt, None)):
                            ps = pps.tile([128, 512], f32, tag="ps",
                                          name=f"ps{pair}{cc}")
                            for d in range(DC):
                                nc.tensor.matmul(
                                    ps[:],
                                    wt[:, d, 128 * pair:128 * pair + 128],
                                    et(d),
                                    start=(d == 0), stop=(d == DC - 1))
                                if d % 2 == 1:
                                    yield
                            # rotate_half comes from a second projection with
                            # host-permuted weight columns (sign in sinM)
                            psr = pps.tile([128, 512], f32, tag="ps",
                                           name=f"pr{pair}{cc}")
                            for d in range(DC):
                                nc.tensor.matmul(
                                    psr[:],
                                    wrt[:, d, 128 * pair:128 * pair + 128],
                                    et(d),
                                    start=(d == 0), stop=(d == DC - 1))
                                if d % 2 == 1 and d < 5:
                                    yield
                            t_t = pjs.tile([128, 512], bf16, tag="t",
                                           bufs=2, name=f"t{pair}{cc}")
                            nc.vector.tensor_tensor(
                                t_t[:], ps[:], cosF[:, cols], OP.mult)
                            u_t = pjs.tile([128, 512], bf16, tag="u",
                                           bufs=2, name=f"u{pair}{cc}")
                            nc.vector.tensor_tensor(
                                u_t[:], psr[:], sinM[:, cols], OP.mult)
                            if dst is not None:
                                nc.gpsimd.tensor_tensor(
                                    dst[:, pair, cols], t_t[:], u_t[:],
                                    OP.add)
                            else:
                                nc.gpsimd.tensor_tensor(
                                    ktA[0:HD, pair, cols], t_t[0:HD, :],
                                    u_t[0:HD, :], OP.add)
                                nc.gpsimd.tensor_tensor(
                                    ktB[HD:128, pair, cols], t_t[HD:128, :],
                                    u_t[HD:128, :], OP.add)
                            yield
                        if with_v:
                            for i in range(4 * cc, 4 * cc + 4):
                                io = 128 * (i % 4)
                                pv = pps.tile([128, 384], f32, tag="ps",
                                              name=f"pv{cc}{i}")
                                for d in range(DC):
                                    nc.tensor.matmul(
                                        pv[:],
                                        et(d)[:, io:io + 128],
                                        wvt[:, d, :],
                                        start=(d == 0), stop=(d == DC - 1))
                                    if d % 2 == 1 and d < 5:
                                        yield
                                nc.vector.tensor_copy(vt[:, i, :, 0:HD], pv[:])
                                yield

                    def attn_win(pair, w0, filler=None):
                        """Attention for both heads of `pair` on q-window w0."""
                        if "noattn" in ablate:
                            consume(filler, 1000)
                            return
                        base = W * w0
                        nj = 4 * w0 + 4
                        CA = cxp.tile([HD + 1, W], f32, tag="C",
                                      name=f"CA{pair}{w0}")
                        CB = cxp.tile([HD + 1, W], f32, tag="C",
                                      name=f"CB{pair}{w0}")

                        def emit_pv(j, et_, qlo):
                            if "pv1" in ablate and j > 0:
                                return
                            off = qlo - base
                            wj = W - off
                            stop = (j == nj - 1) or "pv1" in ablate
                            for hx, C in ((0, CA), (1, CB)):
                                nc.tensor.matmul(
                                    C[:, off:W],
                                    vt[:, j, 2 * pair + hx, :],
                                    et_[:, hx, 0:wj],
                                    start=(j == 0), stop=stop)

                        pend = None
                        for j in range(nj):
                            qlo = max(base, 128 * j)
                            wj = base + W - qlo
                            kk = slice(128 * j, 128 * j + 128)
                            sc = scp.tile([128, 2, W], f32, tag="sc",
                                          name=f"sc{pair}{w0}{j}")
                            diag = qlo == 128 * j
                            for hx, ktX in ((0, ktA), (1, ktB)):
                                nc.tensor.matmul(
                                    sc[:, hx, 0:wj],
                                    ktX[:, pair, kk],
                                    qt[:, pair, qlo:qlo + wj],
                                    start=True, stop=not diag)
                                if diag:   # add -1e4 above the diagonal so
                                    # exp underflows to exact zeros
                                    nc.tensor.matmul(
                                        sc[:, hx, 0:128], msk[:], idn[:],
                                        start=False, stop=True)
                            if "noexp" in ablate:
                                et_ = etc_
                            else:
                                et_ = asb.tile([128, 2, W], f32r, tag="ex",
                                               bufs=3,
                                               name=f"ex{pair}{w0}{j}")
                                nc.scalar.activation(
                                    et_[:, :, 0:wj], sc[:, :, 0:wj], AF.Exp,
                                    scale=0.125)
                            # software pipeline: PV runs one j behind so
                            # PE's in-order queue never waits on exp_j
                            if pend is not None:
                                emit_pv(*pend)
                            pend = (j, et_, qlo)
                            # slip a quantum of projection/output work into
                            # the PE queue behind this step's matmuls
                            consume(filler, 1)
                        emit_pv(*pend)

                        cs = slice(base, base + W)
                        rec = asb.tile([128, W], bf16, tag="rec", bufs=2,
                                       name=f"rc{pair}{w0}")
                        for hx, C in ((0, CA), (1, CB)):
                            po = HD * hx
                            nc.vector.tensor_copy(cxt[po:po + HD, pair, cs],
                                                  C[0:HD, :])
                            rr = asb.tile([1, W], bf16, tag="rr", bufs=2,
                                          name=f"rr{pair}{w0}{hx}")
                            nc.vector.reciprocal(rr[:], C[HD:HD + 1, :])
                            nc.sync.dma_start(
                                rec[po:po + HD, :],
                                rr[0:1, None, :].to_broadcast([1, HD, W]))
                            nc.vector.tensor_tensor(
                                cxt[po:po + HD, pair, cs],
                                cxt[po:po + HD, pair, cs],
                                rec[po:po + HD, :], OP.mult)
                        consume(filler, 1000)   # drain

                    def out_chunk(i, osp):
                        op_ = osp.tile([128, D], f32, tag="op", name=f"op{i}")
                        ss = slice(128 * i, 128 * i + 128)
                        for pair in range(PAIRS):
                            for c0 in range(0, D, 512):
                                cw = min(512, D - c0)
                                nc.tensor.matmul(
                                    op_[:, c0:c0 + cw],
                                    cxt[:, pair, ss],
                                    wot[:, pair, c0:c0 + cw],
                                    start=(pair == 0),
                                    stop=(pair == PAIRS - 1))
                        ot = asb.tile([128, D], f32, tag="ot", bufs=3,
                                      name=f"ot{i}")
                        nc.vector.tensor_copy(ot[:], op_[:])
                        eng = nc.sync if i % 2 == 0 else nc.scalar
                        eng.dma_start(o[ss, :], ot[:])

                    def out_gen(lo, hi, osp):
                        for i in range(lo, hi):
                            op_ = osp.tile([128, D], f32, tag="op",
                                           name=f"op{i}")
                            ss = slice(128 * i, 128 * i + 128)
                            for pair in range(PAIRS):
                                for c0 in range(0, D, 512):
                                    cw = min(512, D - c0)
                                    nc.tensor.matmul(
                                        op_[:, c0:c0 + cw],
                                        cxt[:, pair, ss],
                                        wot[:, pair, c0:c0 + cw],
                                        start=(pair == 0),
                                        stop=(pair == PAIRS - 1))
                                yield
                            ot = asb.tile([128, D], f32, tag="ot", bufs=3,
                                          name=f"ot{i}")
                            nc.vector.tensor_copy(ot[:], op_[:])
                            eng = nc.sync if i % 2 == 0 else nc.scalar
                            eng.dma_start(o[ss, :], ot[:])
                            yield

                    # weights/tables on the ScalarE DMA queue so the eT
                    # stream (SP queue) starts immediately
                    wqt = pjs.tile([128, DC, 384], bf16)
                    nc.scalar.dma_start(
                        wqt[:], wq[:].rearrange("(n p) m -> p n m", p=128))
                    wkt = pjs.tile([128, DC, 384], bf16)
                    nc.scalar.dma_start(
                        wkt[:], wk[:].rearrange("(n p) m -> p n m", p=128))
                    wqrt = pjs.tile([128, DC, 384], bf16)
                    nc.scalar.dma_start(
                        wqrt[:], wqr[:].rearrange("(n p) m -> p n m", p=128))
                    wkrt = pjs.tile([128, DC, 384], bf16)
                    nc.scalar.dma_start(
                        wkrt[:], wkr[:].rearrange("(n p) m -> p n m", p=128))
                    wvt = pjs.tile([128, DC, 384], bf16)
                    nc.scalar.dma_start(
                        wvt[:], wv[:].rearrange("(n p) m -> p n m", p=128))
                    nc.scalar.dma_start(cosF[:], cosF_d[:])
                    nc.scalar.dma_start(sinM[:], sinM_d[:])
                    nc.scalar.dma_start(msk[:], mask_d[:])
                    nc.scalar.dma_start(idn[:], idn_d[:])
                    nc.sync.dma_start(
                        wot[:], wo[:].rearrange("(n p) m -> p n m", p=128))

                    # pipeline: attention windows are the ACT-bound backbone;
                    # projection and output-projection matmuls are drip-fed
                    # between attention steps so exp never stalls behind
                    # bulk PE work. win(p, w) needs pair-p chunks 0..w only.
                    from itertools import chain as _chain

                    def g(pair, cc):
                        return proj_chunk(pair, cc, pair == 0, wqt, wkt, wvt)

                    consume(g(0, 0), 1000)        # eager: feeds win(0, 0)
                    attn_win(0, 0, g(0, 1))
                    attn_win(0, 1, g(0, 2))
                    attn_win(0, 2, g(0, 3))
                    attn_win(0, 3, g(1, 0))
                    attn_win(1, 0, g(1, 1))
                    attn_win(1, 1, g(1, 2))
                    attn_win(1, 2, g(1, 3))
                    attn_win(1, 3, _chain(g(2, 0), g(2, 1)))
                    attn_win(2, 0, _chain(g(2, 2), g(2, 3)))
                    # projection PSUM banks are dead now — recycle for the
                    # output projection so it overlaps pair-2 attention
                    pps_cm.__exit__(None, None, None)
                    osp_cm = tc.tile_pool(name="osp", bufs=1, space="PSUM")
                    osp = osp_cm.__enter__()
                    attn_win(2, 1, out_gen(0, 4, osp))
                    attn_win(2, 2, out_gen(4, 8, osp))
                    attn_win(2, 3, out_gen(8, 12, osp))
                    osp_cm.__exit__(None, None, None)
                    cxp_cm.__exit__(None, None, None)
                    scp_cm.__exit__(None, None, None)
                    osp2_cm = tc.tile_pool(name="osp2", bufs=3, space="PSUM")
                    osp2 = osp2_cm.__enter__()
                    for i in range(12, 16):
                        out_chunk(i, osp2)
                    osp2_cm.__exit__(None, None, None)
                    if dbg:
                        nc.sync.dma_start(qtd[:], qt[:])
                        nc.sync.dma_start(ktd[0:HD], ktA[0:HD])
                        nc.sync.dma_start(ktd[HD:128], ktB[HD:128])
                        nc.sync.dma_start(vtd[:], vt[:])
                        nc.sync.dma_start(cxtd[:], cxt[:])

    nc.compile()
    return nc


def _get_program(reps=1, ablate=()):
    key = (reps, tuple(ablate))
    if key not in _CACHE:
        _CACHE[key] = _build_program(reps, ablate=ablate)
    return _CACHE[key]


_ROT = np.arange(384) ^ 32


def make_in_maps(embeds, Wq, Wk, Wv, Wo):
    import ml_dtypes
    bf16 = ml_dtypes.bfloat16
    cosF, sinM = _rope_tables()
    cosF, sinM = cosF.astype(bf16), sinM.astype(bf16)
    mask = np.where(np.arange(128)[None, :] > np.arange(128)[:, None],
                    -1e4, 0.0).astype(bf16)
    idn = np.eye(128).astype(bf16)
    eTs = [np.ascontiguousarray(embeds[b].T).astype(bf16) for b in range(B)]
    in_maps = []
    for c in range(N_CORES):
        b, hg = c // 2, c % 2
        hs = slice(hg * 384, hg * 384 + 384)
        in_maps.append({
            "eT": eTs[b],
            "wq": np.ascontiguousarray(Wq[hs].T).astype(bf16),
            "wk": np.ascontiguousarray(Wk[hs].T).astype(bf16),
            "wqr": np.ascontiguousarray(Wq[hs].T[:, _ROT]).astype(bf16),
            "wkr": np.ascontiguousarray(Wk[hs].T[:, _ROT]).astype(bf16),
            "wv": np.ascontiguousarray(Wv[hs].T).astype(bf16),
            "wo": np.ascontiguousarray(Wo[:, hs].T).astype(bf16),
            "cosF": cosF, "sinM": sinM, "mask": mask,
            "idn": idn,
        })
    return in_maps


def kernel(embeds, Wq, Wk, Wv, Wo, bo):
    from concourse.bass_utils import run_bass_kernel_spmd

    embeds = np.asarray(embeds, np.float32)
    Wq = np.asarray(Wq, np.float32)
    Wk = np.asarray(Wk, np.float32)
    Wv = np.asarray(Wv, np.float32)
    Wo = np.asarray(Wo, np.float32)
    bo = np.asarray(bo, np.float32)

    nc = _get_program()
    in_maps = make_in_maps(embeds, Wq, Wk, Wv, Wo)
    res = run_bass_kernel_spmd(nc, in_maps, list(range(N_CORES))).results
    out = np.empty((B, S, D), np.float32)
    for b in range(B):
        out[b] = res[2 * b]["o"] + res[2 * b + 1]["o"] + bo
    return out
